# revision 15
# baseline (speedup 1.0000x reference)
"""Trainium2 Bass kernel for 2-hop MixHop GCN (nn_Mixhop).

Strategy (8 NeuronCores, node sharding):
  h = x @ W1 (+b1);  GCN norm folded into row scales:
      g = dinv * h;  y[d] = dinv[d] * sum_{e: src->d} g[src]
  Per hop the fp16 g-table is assembled with TWO sub-AllGathers (each core
  contributes 4096 rows per sub) so each collective overlaps with compute
  and each 32768-row sub-table is addressable with int16 gather indices.
  Self-loop edges never enter the gather stream: their contribution is a
  per-window identity matmul against an SBUF-resident copy of g (gtsb).
  Remaining edges: per-edge dma_gather of source rows, segment-sum via PE
  matmuls with host-built one-hot fp8 "S" matrices (PSUM accumulation per
  128-dst window).  relu'd mats are PE-transposed into matsT for the final
  lin2 (@W2) + log_softmax.
"""

import os
import sys

sys.path.insert(0, "/opt/trn_rl_repo")

import numpy as np

import concourse.bacc as bacc
import concourse.bass as bass
import concourse.mybir as mybir
import concourse.tile as tile
from concourse.bass_utils import run_bass_kernel_spmd

F32 = mybir.dt.float32
F16 = mybir.dt.float16
FP8 = mybir.dt.float8e4
I16 = mybir.dt.int16
NP_FP8 = mybir.dt.np(FP8)
NP_F16 = np.float16

N_CORES = 8
WIN = 128          # dst nodes per PSUM window
CHUNK = 128        # edges per matmul chunk
WG = 4             # windows per gather group
NSUB = 2           # sub-AllGathers per hop (sub-table = NN/NSUB rows)

LAST_EXEC_NS = None
LAST_RESULTS = None


def _preprocess(x, edge_index, W1, b1, W2, b2):
    """Build the chunk plan (program-level constants, max over cores) and
    per-core input arrays."""
    n_nodes, d_in = x.shape
    hid = W1.shape[1]
    ncls = W2.shape[1]
    nmat = W2.shape[0] // hid
    assert n_nodes % (N_CORES * WIN) == 0
    NLOC = n_nodes // N_CORES
    NW = NLOC // WIN
    assert NW % WG == 0
    NG = NW // WG
    KIN = d_in // 128
    assert d_in % 128 == 0 and hid == 128
    QROWS = NLOC // NSUB           # local rows contributed per sub-AG
    SUBN = n_nodes // NSUB         # rows per assembled sub-table
    assert SUBN <= 32768           # int16 index range

    # data edges all flow through the gather path (including any src==dst
    # pairs in the data); only the synthetic self-loops from _gcn_norm are
    # handled by the per-window identity matmul against gtsb.
    src = np.asarray(edge_index[0], dtype=np.int64)
    dst = np.asarray(edge_index[1], dtype=np.int64)

    deg = (np.bincount(dst, minlength=n_nodes) + 1).astype(np.float32)
    dinv = (1.0 / np.sqrt(deg)).astype(np.float32)

    core = dst // NLOC
    w_of = (dst % NLOC) // WIN
    sub_of = (src % NLOC) // QROWS
    tidx = (src // NLOC) * QROWS + (src % QROWS)
    dloc = (dst % WIN).astype(np.int64)

    # counts per (core, window, sub) -> program chunk counts = max over cores
    key = (core * NW + w_of) * NSUB + sub_of
    cnt = np.bincount(key, minlength=N_CORES * NW * NSUB)
    cnt = cnt.reshape(N_CORES, NW, NSUB)
    chunks_pc = -(-cnt // CHUNK)  # ceil-div per core
    C = chunks_pc.max(axis=0)     # [NW, NSUB] max over cores
    C0, C1 = C[:, 0].copy(), C[:, 1].copy()
    CW = C0 + C1

    # group-level layout: gather-call column order per group:
    #   [s0(w0) s0(w1) s0(w2) s0(w3) | s1(w0) .. s1(w3)]
    NS0g = np.array([C0[g * WG:(g + 1) * WG].sum() for g in range(NG)])
    NS1g = np.array([C1[g * WG:(g + 1) * WG].sum() for g in range(NG)])
    NCOLSg = NS0g + NS1g
    gbase = np.concatenate([[0], np.cumsum(NCOLSg)[:-1]])  # col base per group
    TOTC = int(NCOLSg.sum())
    MAXG = int(NCOLSg.max())

    col0 = np.zeros(NW, np.int64)  # within-group col offset of window's chunks
    col1 = np.zeros(NW, np.int64)
    for g in range(NG):
        a0 = a1 = 0
        for w in range(g * WG, (g + 1) * WG):
            col0[w] = a0
            a0 += C0[w]
            col1[w] = a1
            a1 += C1[w]

    # flat gather-slot base for (w, sub)
    slotbase = np.zeros((NW, NSUB), np.int64)
    for w in range(NW):
        g = w // WG
        slotbase[w, 0] = (gbase[g] + col0[w]) * CHUNK
        slotbase[w, 1] = (gbase[g] + NS0g[g] + col1[w]) * CHUNK
    TOTSLOTS = TOTC * CHUNK

    # S data col base per window (sdat layout: per-window [s0 chunks | s1])
    soff = np.concatenate([[0], np.cumsum(CW)[:-1]]) * CHUNK
    CMAXW = int(CW.max())

    one_fp8 = np.float32(1.0).astype(NP_FP8).view(np.uint8)

    plan = dict(
        n_nodes=n_nodes, NLOC=NLOC, QROWS=QROWS, SUBN=SUBN,
        NW=NW, NG=NG, KIN=KIN,
        hid=hid, ncls=ncls, nmat=nmat,
        C0=C0, C1=C1, CW=CW, NS0g=NS0g, NS1g=NS1g, gbase=gbase,
        col0=col0, col1=col1, soff=soff,
        TOTC=TOTC, TOTSLOTS=TOTSLOTS, MAXG=MAXG, CMAXW=CMAXW,
        has_b1=bool(np.any(b1 != 0)), has_b2=bool(np.any(b2 != 0)),
    )

    in_maps = []
    for p in range(N_CORES):
        sel = core == p
        s_p, w_p, sub_p, dl_p, t_p = (src[sel], w_of[sel], sub_of[sel],
                                      dloc[sel], tidx[sel])
        k = w_p * NSUB + sub_p
        order = np.argsort(k, kind="stable")
        ks = k[order]
        gcnt = np.bincount(ks, minlength=NW * NSUB)
        run_start = np.cumsum(gcnt) - gcnt
        run_pos = np.arange(len(ks)) - np.repeat(run_start, gcnt)
        slots = slotbase.reshape(-1)[ks] + run_pos

        idx_flat = np.zeros(TOTSLOTS, np.int16)
        idx_flat[slots] = t_p[order].astype(np.int16)
        idx16 = idx_flat.reshape(TOTSLOTS // 16, 16).T  # [16, S/16]
        idx_arr = np.tile(idx16, (8, 1)).copy()         # [128, S/16]

        # S one-hot: row = pos-in-chunk, col = window-S-col
        su8 = np.zeros((CHUNK, TOTC * CHUNK), np.uint8)
        c_in_list = run_pos // CHUNK
        pos = run_pos % CHUNK
        w_o = w_p[order]
        scol = (soff[w_o]
                + (c_in_list + np.where(sub_p[order] == 1, C0[w_o], 0))
                * CHUNK + dl_p[order])
        su8[pos, scol] = one_fp8
        s_arr = su8.view(NP_FP8)

        x_p = np.asarray(x[p * NLOC:(p + 1) * NLOC], dtype=np.float32)
        xt = np.ascontiguousarray(
            x_p.reshape(NW, 128, KIN, 128).transpose(0, 3, 2, 1)
            .reshape(NW, 128, KIN * 128))
        dinv_p = np.ascontiguousarray(
            dinv[p * NLOC:(p + 1) * NLOC].reshape(NW, 128).T)

        m = {
            "xt": xt.astype(NP_F16),
            "w1": np.ascontiguousarray(
                np.asarray(W1, np.float32).reshape(KIN, 128, hid)
                .transpose(1, 0, 2).reshape(128, KIN * hid)).astype(NP_F16),
            "w2": np.ascontiguousarray(
                np.asarray(W2, np.float32).reshape(nmat, hid, ncls)
                .astype(NP_F16).transpose(1, 0, 2).reshape(hid, nmat * ncls)),
            "dinv": dinv_p,
            "dinv2": (dinv_p * dinv_p),
            "idx": idx_arr,
            "sdat": s_arr,
            "ident": np.eye(128, dtype=NP_F16),
        }
        if plan["has_b1"]:
            m["b1bc"] = np.tile(np.asarray(b1, np.float32)[None, :], (128, 1))
        if plan["has_b2"]:
            m["b2bc"] = np.tile(np.asarray(b2, np.float32)[None, :], (128, 1))
        in_maps.append(m)
    return plan, in_maps


def _build(plan):
    P = plan
    NLOC, NW, NG, KIN = P["NLOC"], P["NW"], P["NG"], P["KIN"]
    HID, NCLS, NMAT = P["hid"], P["ncls"], P["nmat"]
    QROWS, SUBN = P["QROWS"], P["SUBN"]
    C0, C1, CW = P["C0"], P["C1"], P["CW"]
    NS0g, NS1g, gbase = P["NS0g"], P["NS1g"], P["gbase"]
    col0, col1, soff = P["col0"], P["col1"], P["soff"]
    MAXG, CMAXW, TOTC, TOTSLOTS = (P["MAXG"], P["CMAXW"], P["TOTC"],
                                   P["TOTSLOTS"])
    WPS = NW // NSUB   # windows per sub (drain target ranges)

    nc = bacc.Bacc("TRN2", target_bir_lowering=False, debug=False,
                   num_devices=N_CORES, num_swdge_queues=4)
    xt_d = nc.dram_tensor("xt", [NW, 128, KIN * 128], F16,
                          kind="ExternalInput")
    w1_d = nc.dram_tensor("w1", [128, KIN * HID], F16, kind="ExternalInput")
    w2_d = nc.dram_tensor("w2", [128, NMAT * NCLS], F16, kind="ExternalInput")
    dinv_d = nc.dram_tensor("dinv", [128, NW], F32, kind="ExternalInput")
    dinv2_d = nc.dram_tensor("dinv2", [128, NW], F32, kind="ExternalInput")
    idx_d = nc.dram_tensor("idx", [128, TOTSLOTS // 16], I16,
                           kind="ExternalInput")
    sdat_d = nc.dram_tensor("sdat", [128, TOTC * CHUNK], FP8,
                            kind="ExternalInput")
    id_d = nc.dram_tensor("ident", [128, 128], F16, kind="ExternalInput")
    b1_d = (nc.dram_tensor("b1bc", [128, HID], F32, kind="ExternalInput")
            if P["has_b1"] else None)
    b2_d = (nc.dram_tensor("b2bc", [128, NCLS], F32, kind="ExternalInput")
            if P["has_b2"] else None)
    y_d = nc.dram_tensor("y", [NLOC, NCLS], F32, kind="ExternalOutput")

    rg = [list(range(N_CORES))]

    with tile.TileContext(nc) as tc:
        # ---- persistent tiles ----
        perm = tc.alloc_tile_pool(name="perm", bufs=1)
        dramp = tc.alloc_tile_pool(name="dramp", bufs=1, space="DRAM")
        w1_sb = perm.tile([128, KIN * HID], F16, name="w1sb")
        w2_sb = perm.tile([128, NMAT * NCLS], F16, name="w2sb")
        dinv_sb = perm.tile([128, NW], F32, name="dinvsb")
        dinv2_sb = perm.tile([128, NW], F32, name="dinv2sb")
        idx_sb = perm.tile([128, TOTSLOTS // 16], I16, name="idxsb")
        id_sb = perm.tile([128, 128], F16, name="idsb")
        matsT = [perm.tile([128, NLOC], F16, name=f"matsT{i}")
                 for i in range(NMAT)]
        gtsb = perm.tile([128, NW * HID], F16, name="gtsb")
        logits = perm.tile([128, NW * NCLS], F32, name="logits")
        ssum = perm.tile([128, NW], F32, name="ssum")
        lsum = perm.tile([128, NW], F32, name="lsum")
        final = perm.tile([128, NW * NCLS], F32, name="final")
        b1_sb = perm.tile([128, HID], F32, name="b1sb") if b1_d else None
        b2_sb = perm.tile([128, NCLS], F32, name="b2sb") if b2_d else None

        gin = [[dramp.tile([QROWS, HID], F16, name=f"gin{h}_{s}")
                for s in range(NSUB)] for h in range(2)]
        gout = [[dramp.tile([SUBN, HID], F16, addr_space="Shared",
                            name=f"gout{h}_{s}")
                 for s in range(NSUB)] for h in range(2)]

        nc.sync.dma_start(out=w1_sb[:], in_=w1_d[:])
        nc.sync.dma_start(out=w2_sb[:], in_=w2_d[:])
        nc.sync.dma_start(out=dinv_sb[:], in_=dinv_d[:])
        nc.sync.dma_start(out=dinv2_sb[:], in_=dinv2_d[:])
        nc.sync.dma_start(out=idx_sb[:], in_=idx_d[:])
        nc.sync.dma_start(out=id_sb[:], in_=id_d[:])
        if b1_d is not None:
            nc.sync.dma_start(out=b1_sb[:], in_=b1_d[:])
        if b2_d is not None:
            nc.sync.dma_start(out=b2_sb[:], in_=b2_d[:])

        with (
            tc.tile_pool(name="xp", bufs=6) as xp,
            tc.tile_pool(name="gp", bufs=8) as gp,
            tc.tile_pool(name="sp", bufs=8) as sp,
            tc.tile_pool(name="dp", bufs=6) as dp,
            tc.tile_pool(name="pp", bufs=1, space="PSUM") as pp,
        ):
            ACT = mybir.ActivationFunctionType

            def drain_window(acc, w, hop):
                """acc: PSUM [128, HID] f32 for window w; hop 0/1/-1 (lin1).

                lin1: h = acc.  hops: h = dinv * acc (acc includes the self
                term via the identity matmul).  g for the next hop's table
                goes to gtsb (SBUF, feeds the self matmul) and to gin (DRAM,
                feeds the sub-AllGather).  mats = relu(h).  All on ScalarE:
                DVE shares an SBUF port with GpSimd (SWDGE) and stalls while
                gathers run."""
                if hop < 0:
                    # lin1 runs before any SWDGE traffic: DVE is free, and
                    # using it lets the sub-AllGathers launch sooner.
                    gt = gtsb[:, w * HID:(w + 1) * HID]
                    nc.vector.tensor_scalar_mul(gt, acc[:],
                                                dinv_sb[:, w:w + 1])
                    nc.sync.dma_start(
                        out=gin[0][w // WPS][(w % WPS) * 128:
                                             (w % WPS + 1) * 128, :],
                        in_=gt)
                    m = dp.tile([128, HID], F16, tag="m")
                    nc.vector.tensor_scalar_max(m[:], acc[:], 0.0)
                    tp = pp.tile([128, 128], F16, tag="tp", bufs=2)
                    nc.tensor.transpose(tp[:], m[:], id_sb[:])
                    nc.vector.tensor_copy(
                        matsT[0][:, w * 128:(w + 1) * 128], tp[:])
                    return
                hscale = dinv_sb[:, w:w + 1]
                if hop < 1:  # produce g for the next hop
                    gt = gtsb[:, w * HID:(w + 1) * HID]
                    nc.scalar.activation(gt, acc[:], ACT.Copy,
                                         scale=dinv2_sb[:, w:w + 1])
                    s = w // WPS
                    r = w % WPS
                    nc.sync.dma_start(
                        out=gin[hop + 1][s][r * 128:(r + 1) * 128, :], in_=gt)
                m = dp.tile([128, HID], F16, tag="m")
                nc.scalar.activation(m[:], acc[:], ACT.Relu, scale=hscale)
                tp = pp.tile([128, 128], F16, tag="tp", bufs=2)
                nc.tensor.transpose(tp[:], m[:], id_sb[:])
                nc.scalar.activation(matsT[hop + 1][:, w * 128:(w + 1) * 128],
                                     tp[:], ACT.Copy)

            # ---- lin1 ----
            for t in range(NW):
                acc = pp.tile([128, HID], F32, tag="acc", bufs=4)
                xtile = xp.tile([128, KIN * 128], F16, tag="xt")
                nc.sync.dma_start(out=xtile[:], in_=xt_d[t])
                for k in range(KIN):
                    nc.tensor.matmul(acc[:], xtile[:, k * 128:(k + 1) * 128],
                                     w1_sb[:, k * HID:(k + 1) * HID],
                                     start=(k == 0), stop=(k == KIN - 1))
                if b1_sb is not None:
                    hb = dp.tile([128, HID], F32, tag="hb")
                    nc.vector.tensor_tensor(hb[:], acc[:], b1_sb[:],
                                            op=mybir.AluOpType.add)
                    drain_window(hb, t, -1)
                else:
                    drain_window(acc, t, -1)

            def lin2_tile(t):
                lg = pp.tile([128, NCLS], F32, tag="lg", bufs=2)
                for mi in range(NMAT):
                    nc.tensor.matmul(lg[:], matsT[mi][:, t * 128:(t + 1) * 128],
                                     w2_sb[:, mi * NCLS:(mi + 1) * NCLS],
                                     start=(mi == 0), stop=(mi == NMAT - 1))
                dst = logits[:, t * NCLS:(t + 1) * NCLS]
                if b2_sb is not None:
                    nc.vector.tensor_tensor(dst, lg[:], b2_sb[:],
                                            op=mybir.AluOpType.add)
                else:
                    nc.vector.tensor_copy(dst, lg[:])

            # ---- hops ----
            no_cc = os.environ.get("MIXHOP_NO_CC", "0") == "1"
            no_gather = os.environ.get("MIXHOP_NO_GATHER", "0") == "1"
            nsplit = int(os.environ.get("MIXHOP_NSPLIT", "2"))
            # sub-0 gather groups issued ahead of each sub-1 collective:
            # must stay < gp bufs or the in-order gpsimd queue deadlocks on
            # G-buffer reuse semaphores.
            PRE = int(os.environ.get("MIXHOP_PRE", "7"))
            AG_EMIT_G = 10  # hop-0 group after which AG(1,0) is emitted
            MAXGS = int(max(NS0g.max(), NS1g.max()))

            def allgather(hop, s):
                if no_cc:
                    nc.sync.dma_start(out=gout[hop][s][0:QROWS, :],
                                      in_=gin[hop][s][:])
                else:
                    nc.gpsimd.collective_compute(
                        "AllGather", mybir.AluOpType.bypass,
                        replica_groups=rg,
                        ins=[gin[hop][s][:]], outs=[gout[hop][s][:]])

            gtiles = {}

            def gather_sub(hop, g, s):
                """Issue the dma_gather calls for (group g, sub s)."""
                G = gp.tile([128, MAXGS * 128], F16, tag="G")
                gtiles[(hop, g, s)] = G
                G3 = G[:].rearrange("p (c e) -> p c e", e=128)
                ns = int(NS0g[g]) if s == 0 else int(NS1g[g])
                if ns == 0:
                    return
                base = (int(gbase[g]) + (0 if s == 0 else int(NS0g[g]))) \
                    * CHUNK
                if no_gather:
                    nc.vector.memset(G[:, :ns * 128], 0.0)
                    return
                table = gout[hop][s]
                q, r = divmod(ns, nsplit)
                off = 0
                for i in range(nsplit):
                    ncols = q + (1 if i < r else 0)
                    if not ncols:
                        continue
                    s0 = base + off * CHUNK
                    nc.gpsimd.dma_gather(
                        G3[:, off:off + ncols, :], table[:],
                        idx_sb[:, s0 // 16:(s0 + ncols * CHUNK) // 16],
                        ncols * CHUNK, ncols * CHUNK, HID,
                        single_packet=False,
                        queue_num=(s * nsplit + i) % 4)
                    off += ncols

            allgather(0, 0)
            for g in range(PRE):
                gather_sub(0, g, 0)
            allgather(0, 1)
            for hop in range(2):
                for g in range(NG):
                    if g >= PRE:
                        gather_sub(hop, g, 0)
                    gather_sub(hop, g, 1)
                    G0 = gtiles[(hop, g, 0)]
                    G0_3 = G0[:].rearrange("p (c e) -> p c e", e=128)
                    G1 = gtiles[(hop, g, 1)]
                    G1_3 = G1[:].rearrange("p (c e) -> p c e", e=128)
                    for w in range(g * WG, (g + 1) * WG):
                        cw = int(CW[w])
                        S = sp.tile([128, CMAXW * 128], FP8, tag="S")
                        nc.sync.dma_start(
                            out=S[:, :cw * 128],
                            in_=sdat_d[:, int(soff[w]):int(soff[w])
                                       + cw * 128])
                        acc = pp.tile([128, HID], F32, tag="acc", bufs=4)
                        # self-loop contribution: acc = I @ g_prev[window w]
                        nc.tensor.matmul(acc[:], id_sb[:],
                                         gtsb[:, w * HID:(w + 1) * HID],
                                         start=True, stop=(cw == 0))
                        ci = 0
                        for c in range(int(C0[w])):
                            nc.tensor.matmul(
                                acc[:], S[:, ci * 128:(ci + 1) * 128],
                                G0_3[:, int(col0[w]) + c, :],
                                start=False, stop=(ci == cw - 1))
                            ci += 1
                        for c in range(int(C1[w])):
                            nc.tensor.matmul(
                                acc[:], S[:, ci * 128:(ci + 1) * 128],
                                G1_3[:, int(col1[w]) + c, :],
                                start=False, stop=(ci == cw - 1))
                            ci += 1
                        drain_window(acc, w, hop)
                        if hop == 1:
                            lin2_tile(w)
                    if hop == 0 and g == AG_EMIT_G:
                        allgather(1, 0)
                    if hop == 0 and g == NG - 1:
                        for g2 in range(PRE):
                            gather_sub(1, g2, 0)
                        allgather(1, 1)

            # ---- log_softmax (lin2 already streamed into hop-1 drains;
            # `final` doubles as the exp scratch before the subtract) ----
            nc.scalar.activation(final[:], logits[:],
                                 mybir.ActivationFunctionType.Exp)
            nc.vector.reduce_sum(
                ssum[:], final[:].rearrange("p (t c) -> p t c", c=NCLS),
                axis=mybir.AxisListType.X)
            nc.scalar.activation(lsum[:], ssum[:],
                                 mybir.ActivationFunctionType.Ln)
            for t in range(NW):
                nc.vector.tensor_scalar_sub(
                    final[:, t * NCLS:(t + 1) * NCLS],
                    logits[:, t * NCLS:(t + 1) * NCLS], lsum[:, t:t + 1])
            nc.sync.dma_start(
                out=y_d[:].rearrange("(t q) c -> q t c", q=128),
                in_=final[:].rearrange("p (t c) -> p t c", c=NCLS))
        perm.release()
        dramp.release()
    nc.compile()
    return nc


def _ensure_ntff_hook():
    """The agent image's antenv lacks axon_hooks; synthesize it so
    run_bass_kernel_spmd(trace=True) can NTFF-profile via the axon .so."""
    import types

    if "antenv.axon_hooks" in sys.modules:
        return
    try:
        from trn_agent_boot.trn_boot import _ntff_profile_via_ctypes
        hook = _ntff_profile_via_ctypes("/opt/axon/libaxon_pjrt.so")
    except Exception:
        hook = None
    mod = types.ModuleType("antenv.axon_hooks")
    mod.get_axon_ntff_profile_hook = lambda: hook
    mod.set_axon_ntff_profile_hook = lambda h: None
    sys.modules["antenv.axon_hooks"] = mod


def kernel(x, edge_index, W1, b1, W2, b2):
    global LAST_EXEC_NS, LAST_RESULTS
    plan, in_maps = _preprocess(x, edge_index, W1, b1, W2, b2)
    nc = _build(plan)
    trace = os.environ.get("MIXHOP_TRACE", "0") == "1"
    if trace:
        _ensure_ntff_hook()
    res = run_bass_kernel_spmd(nc, in_maps, list(range(N_CORES)), trace=trace)
    LAST_EXEC_NS = res.exec_time_ns
    LAST_RESULTS = res
    out = np.concatenate([res.results[p]["y"] for p in range(N_CORES)], axis=0)
    return out.astype(np.float32)


# revision 17
# speedup vs baseline: 1.0039x; 1.0039x over previous
"""Trainium2 Bass kernel for 2-hop MixHop GCN (nn_Mixhop).

Strategy (8 NeuronCores, node sharding):
  h = x @ W1 (+b1);  GCN norm folded into row scales:
      g = dinv * h;  y[d] = dinv[d] * sum_{e: src->d} g[src]
  Per hop the fp16 g-table is assembled with TWO sub-AllGathers (each core
  contributes 4096 rows per sub) so each collective overlaps with compute
  and each 32768-row sub-table is addressable with int16 gather indices.
  Self-loop edges never enter the gather stream: their contribution is a
  per-window identity matmul against an SBUF-resident copy of g (gtsb).
  Remaining edges: per-edge dma_gather of source rows, segment-sum via PE
  matmuls with host-built one-hot fp8 "S" matrices (PSUM accumulation per
  128-dst window).  relu'd mats are PE-transposed into matsT for the final
  lin2 (@W2) + log_softmax.
"""

import os
import sys

sys.path.insert(0, "/opt/trn_rl_repo")

import numpy as np

import concourse.bacc as bacc
import concourse.bass as bass
import concourse.mybir as mybir
import concourse.tile as tile
from concourse.bass_utils import run_bass_kernel_spmd

F32 = mybir.dt.float32
F16 = mybir.dt.float16
FP8 = mybir.dt.float8e4
I16 = mybir.dt.int16
NP_FP8 = mybir.dt.np(FP8)
NP_F16 = np.float16

N_CORES = 8
WIN = 128          # dst nodes per PSUM window
CHUNK = 128        # edges per matmul chunk
WG = 4             # windows per gather group
NSUB = 2           # sub-AllGathers per hop (sub-table = NN/NSUB rows)

LAST_EXEC_NS = None
LAST_RESULTS = None


def _preprocess(x, edge_index, W1, b1, W2, b2):
    """Build the chunk plan (program-level constants, max over cores) and
    per-core input arrays."""
    n_nodes, d_in = x.shape
    hid = W1.shape[1]
    ncls = W2.shape[1]
    nmat = W2.shape[0] // hid
    assert n_nodes % (N_CORES * WIN) == 0
    NLOC = n_nodes // N_CORES
    NW = NLOC // WIN
    assert NW % WG == 0
    NG = NW // WG
    KIN = d_in // 128
    assert d_in % 128 == 0 and hid == 128
    QROWS = NLOC // NSUB           # local rows contributed per sub-AG
    SUBN = n_nodes // NSUB         # rows per assembled sub-table
    assert SUBN <= 32768           # int16 index range

    # data edges all flow through the gather path (including any src==dst
    # pairs in the data); only the synthetic self-loops from _gcn_norm are
    # handled by the per-window identity matmul against gtsb.
    src = np.asarray(edge_index[0], dtype=np.int64)
    dst = np.asarray(edge_index[1], dtype=np.int64)

    deg = (np.bincount(dst, minlength=n_nodes) + 1).astype(np.float32)
    dinv = (1.0 / np.sqrt(deg)).astype(np.float32)

    core = dst // NLOC
    w_of = (dst % NLOC) // WIN
    sub_of = (src % NLOC) // QROWS
    tidx = (src // NLOC) * QROWS + (src % QROWS)
    dloc = (dst % WIN).astype(np.int64)

    # counts per (core, window, sub) -> program chunk counts = max over cores
    key = (core * NW + w_of) * NSUB + sub_of
    cnt = np.bincount(key, minlength=N_CORES * NW * NSUB)
    cnt = cnt.reshape(N_CORES, NW, NSUB)
    chunks_pc = -(-cnt // CHUNK)  # ceil-div per core
    C = chunks_pc.max(axis=0)     # [NW, NSUB] max over cores
    C0, C1 = C[:, 0].copy(), C[:, 1].copy()
    CW = C0 + C1

    # group-level layout: gather-call column order per group:
    #   [s0(w0) s0(w1) s0(w2) s0(w3) | s1(w0) .. s1(w3)]
    NS0g = np.array([C0[g * WG:(g + 1) * WG].sum() for g in range(NG)])
    NS1g = np.array([C1[g * WG:(g + 1) * WG].sum() for g in range(NG)])
    NCOLSg = NS0g + NS1g
    gbase = np.concatenate([[0], np.cumsum(NCOLSg)[:-1]])  # col base per group
    TOTC = int(NCOLSg.sum())
    MAXG = int(NCOLSg.max())

    col0 = np.zeros(NW, np.int64)  # within-group col offset of window's chunks
    col1 = np.zeros(NW, np.int64)
    for g in range(NG):
        a0 = a1 = 0
        for w in range(g * WG, (g + 1) * WG):
            col0[w] = a0
            a0 += C0[w]
            col1[w] = a1
            a1 += C1[w]

    # flat gather-slot base for (w, sub)
    slotbase = np.zeros((NW, NSUB), np.int64)
    for w in range(NW):
        g = w // WG
        slotbase[w, 0] = (gbase[g] + col0[w]) * CHUNK
        slotbase[w, 1] = (gbase[g] + NS0g[g] + col1[w]) * CHUNK
    TOTSLOTS = TOTC * CHUNK

    # S data col base per window (sdat layout: per-window [s0 chunks | s1])
    soff = np.concatenate([[0], np.cumsum(CW)[:-1]]) * CHUNK
    CMAXW = int(CW.max())

    one_fp8 = np.float32(1.0).astype(NP_FP8).view(np.uint8)

    plan = dict(
        n_nodes=n_nodes, NLOC=NLOC, QROWS=QROWS, SUBN=SUBN,
        NW=NW, NG=NG, KIN=KIN,
        hid=hid, ncls=ncls, nmat=nmat,
        C0=C0, C1=C1, CW=CW, NS0g=NS0g, NS1g=NS1g, gbase=gbase,
        col0=col0, col1=col1, soff=soff,
        TOTC=TOTC, TOTSLOTS=TOTSLOTS, MAXG=MAXG, CMAXW=CMAXW,
        has_b1=bool(np.any(b1 != 0)), has_b2=bool(np.any(b2 != 0)),
    )

    in_maps = []
    for p in range(N_CORES):
        sel = core == p
        s_p, w_p, sub_p, dl_p, t_p = (src[sel], w_of[sel], sub_of[sel],
                                      dloc[sel], tidx[sel])
        k = w_p * NSUB + sub_p
        order = np.argsort(k, kind="stable")
        ks = k[order]
        gcnt = np.bincount(ks, minlength=NW * NSUB)
        run_start = np.cumsum(gcnt) - gcnt
        run_pos = np.arange(len(ks)) - np.repeat(run_start, gcnt)
        slots = slotbase.reshape(-1)[ks] + run_pos

        idx_flat = np.zeros(TOTSLOTS, np.int16)
        idx_flat[slots] = t_p[order].astype(np.int16)
        idx16 = idx_flat.reshape(TOTSLOTS // 16, 16).T  # [16, S/16]
        idx_arr = np.tile(idx16, (8, 1)).copy()         # [128, S/16]

        # S one-hot: row = pos-in-chunk, col = window-S-col
        su8 = np.zeros((CHUNK, TOTC * CHUNK), np.uint8)
        c_in_list = run_pos // CHUNK
        pos = run_pos % CHUNK
        w_o = w_p[order]
        scol = (soff[w_o]
                + (c_in_list + np.where(sub_p[order] == 1, C0[w_o], 0))
                * CHUNK + dl_p[order])
        su8[pos, scol] = one_fp8
        s_arr = su8.view(NP_FP8)

        x_p = np.asarray(x[p * NLOC:(p + 1) * NLOC], dtype=np.float32)
        xt = np.ascontiguousarray(
            x_p.reshape(NW, 128, KIN, 128).transpose(0, 3, 2, 1)
            .reshape(NW, 128, KIN * 128))
        dinv_p = np.ascontiguousarray(
            dinv[p * NLOC:(p + 1) * NLOC].reshape(NW, 128).T)

        m = {
            "xt": xt.astype(NP_F16),
            "w1": np.ascontiguousarray(
                np.asarray(W1, np.float32).reshape(KIN, 128, hid)
                .transpose(1, 0, 2).reshape(128, KIN * hid)).astype(NP_F16),
            "w2": np.ascontiguousarray(
                np.asarray(W2, np.float32).reshape(nmat, hid, ncls)
                .astype(NP_F16).transpose(1, 0, 2).reshape(hid, nmat * ncls)),
            "dinv": dinv_p,
            "dinv2": (dinv_p * dinv_p),
            "idx": idx_arr,
            "sdat": s_arr,
            "ident": np.eye(128, dtype=NP_F16),
        }
        if plan["has_b1"]:
            m["b1bc"] = np.tile(np.asarray(b1, np.float32)[None, :], (128, 1))
        if plan["has_b2"]:
            m["b2bc"] = np.tile(np.asarray(b2, np.float32)[None, :], (128, 1))
        in_maps.append(m)
    return plan, in_maps


def _build(plan):
    P = plan
    NLOC, NW, NG, KIN = P["NLOC"], P["NW"], P["NG"], P["KIN"]
    HID, NCLS, NMAT = P["hid"], P["ncls"], P["nmat"]
    QROWS, SUBN = P["QROWS"], P["SUBN"]
    C0, C1, CW = P["C0"], P["C1"], P["CW"]
    NS0g, NS1g, gbase = P["NS0g"], P["NS1g"], P["gbase"]
    col0, col1, soff = P["col0"], P["col1"], P["soff"]
    MAXG, CMAXW, TOTC, TOTSLOTS = (P["MAXG"], P["CMAXW"], P["TOTC"],
                                   P["TOTSLOTS"])
    WPS = NW // NSUB   # windows per sub (drain target ranges)

    nc = bacc.Bacc("TRN2", target_bir_lowering=False, debug=False,
                   num_devices=N_CORES, num_swdge_queues=4)
    xt_d = nc.dram_tensor("xt", [NW, 128, KIN * 128], F16,
                          kind="ExternalInput")
    w1_d = nc.dram_tensor("w1", [128, KIN * HID], F16, kind="ExternalInput")
    w2_d = nc.dram_tensor("w2", [128, NMAT * NCLS], F16, kind="ExternalInput")
    dinv_d = nc.dram_tensor("dinv", [128, NW], F32, kind="ExternalInput")
    dinv2_d = nc.dram_tensor("dinv2", [128, NW], F32, kind="ExternalInput")
    idx_d = nc.dram_tensor("idx", [128, TOTSLOTS // 16], I16,
                           kind="ExternalInput")
    sdat_d = nc.dram_tensor("sdat", [128, TOTC * CHUNK], FP8,
                            kind="ExternalInput")
    id_d = nc.dram_tensor("ident", [128, 128], F16, kind="ExternalInput")
    b1_d = (nc.dram_tensor("b1bc", [128, HID], F32, kind="ExternalInput")
            if P["has_b1"] else None)
    b2_d = (nc.dram_tensor("b2bc", [128, NCLS], F32, kind="ExternalInput")
            if P["has_b2"] else None)
    y_d = nc.dram_tensor("y", [NLOC, NCLS], F32, kind="ExternalOutput")

    rg = [list(range(N_CORES))]

    with tile.TileContext(nc) as tc:
        # ---- persistent tiles ----
        perm = tc.alloc_tile_pool(name="perm", bufs=1)
        dramp = tc.alloc_tile_pool(name="dramp", bufs=1, space="DRAM")
        w1_sb = perm.tile([128, KIN * HID], F16, name="w1sb")
        w2_sb = perm.tile([128, NMAT * NCLS], F16, name="w2sb")
        dinv_sb = perm.tile([128, NW], F32, name="dinvsb")
        dinv2_sb = perm.tile([128, NW], F32, name="dinv2sb")
        idx_sb = perm.tile([128, TOTSLOTS // 16], I16, name="idxsb")
        id_sb = perm.tile([128, 128], F16, name="idsb")
        matsT = [perm.tile([128, NLOC], F16, name=f"matsT{i}")
                 for i in range(NMAT)]
        gtsb = perm.tile([128, NW * HID], F16, name="gtsb")
        logits = perm.tile([128, NW * NCLS], F32, name="logits")
        ssum = perm.tile([128, NW], F32, name="ssum")
        lsum = perm.tile([128, NW], F32, name="lsum")
        final = perm.tile([128, NW * NCLS], F32, name="final")
        b1_sb = perm.tile([128, HID], F32, name="b1sb") if b1_d else None
        b2_sb = perm.tile([128, NCLS], F32, name="b2sb") if b2_d else None

        gin = [[dramp.tile([QROWS, HID], F16, name=f"gin{h}_{s}")
                for s in range(NSUB)] for h in range(2)]
        gout = [[dramp.tile([SUBN, HID], F16, addr_space="Shared",
                            name=f"gout{h}_{s}")
                 for s in range(NSUB)] for h in range(2)]

        nc.sync.dma_start(out=w1_sb[:], in_=w1_d[:])
        nc.sync.dma_start(out=w2_sb[:], in_=w2_d[:])
        nc.sync.dma_start(out=dinv_sb[:], in_=dinv_d[:])
        nc.sync.dma_start(out=dinv2_sb[:], in_=dinv2_d[:])
        nc.sync.dma_start(out=idx_sb[:], in_=idx_d[:])
        nc.sync.dma_start(out=id_sb[:], in_=id_d[:])
        if b1_d is not None:
            nc.sync.dma_start(out=b1_sb[:], in_=b1_d[:])
        if b2_d is not None:
            nc.sync.dma_start(out=b2_sb[:], in_=b2_d[:])

        with (
            tc.tile_pool(name="xp", bufs=6) as xp,
            tc.tile_pool(name="gp", bufs=8) as gp,
            tc.tile_pool(name="sp", bufs=8) as sp,
            tc.tile_pool(name="dp", bufs=6) as dp,
            tc.tile_pool(name="pp", bufs=1, space="PSUM") as pp,
        ):
            ACT = mybir.ActivationFunctionType

            def drain_window(acc, w, hop):
                """acc: PSUM [128, HID] f32 for window w; hop 0/1/-1 (lin1).

                lin1: h = acc.  hops: h = dinv * acc (acc includes the self
                term via the identity matmul).  g for the next hop's table
                goes to gtsb (SBUF, feeds the self matmul) and to gin (DRAM,
                feeds the sub-AllGather).  mats = relu(h).  All on ScalarE:
                DVE shares an SBUF port with GpSimd (SWDGE) and stalls while
                gathers run."""
                if hop < 0:
                    # lin1 runs before any SWDGE traffic: DVE is free, and
                    # using it lets the sub-AllGathers launch sooner.
                    gt = gtsb[:, w * HID:(w + 1) * HID]
                    nc.vector.tensor_scalar_mul(gt, acc[:],
                                                dinv_sb[:, w:w + 1])
                    nc.sync.dma_start(
                        out=gin[0][w // WPS][(w % WPS) * 128:
                                             (w % WPS + 1) * 128, :],
                        in_=gt)
                    m = dp.tile([128, HID], F16, tag="m")
                    nc.vector.tensor_scalar_max(m[:], acc[:], 0.0)
                    tp = pp.tile([128, 128], F16, tag="tp", bufs=2)
                    nc.tensor.transpose(tp[:], m[:], id_sb[:])
                    nc.vector.tensor_copy(
                        matsT[0][:, w * 128:(w + 1) * 128], tp[:])
                    return
                hscale = dinv_sb[:, w:w + 1]
                if hop < 1:  # produce g for the next hop
                    gt = gtsb[:, w * HID:(w + 1) * HID]
                    nc.scalar.activation(gt, acc[:], ACT.Copy,
                                         scale=dinv2_sb[:, w:w + 1])
                    s = w // WPS
                    r = w % WPS
                    nc.sync.dma_start(
                        out=gin[hop + 1][s][r * 128:(r + 1) * 128, :], in_=gt)
                m = dp.tile([128, HID], F16, tag="m")
                nc.scalar.activation(m[:], acc[:], ACT.Relu, scale=hscale)
                tp = pp.tile([128, 128], F16, tag="tp", bufs=2)
                nc.tensor.transpose(tp[:], m[:], id_sb[:])
                nc.scalar.activation(matsT[hop + 1][:, w * 128:(w + 1) * 128],
                                     tp[:], ACT.Copy)

            # ---- lin1 ----
            for t in range(NW):
                acc = pp.tile([128, HID], F32, tag="acc", bufs=4)
                xtile = xp.tile([128, KIN * 128], F16, tag="xt")
                nc.sync.dma_start(out=xtile[:], in_=xt_d[t])
                for k in range(KIN):
                    nc.tensor.matmul(acc[:], xtile[:, k * 128:(k + 1) * 128],
                                     w1_sb[:, k * HID:(k + 1) * HID],
                                     start=(k == 0), stop=(k == KIN - 1))
                if b1_sb is not None:
                    hb = dp.tile([128, HID], F32, tag="hb")
                    nc.vector.tensor_tensor(hb[:], acc[:], b1_sb[:],
                                            op=mybir.AluOpType.add)
                    drain_window(hb, t, -1)
                else:
                    drain_window(acc, t, -1)

            def lin2_tile(t):
                lg = pp.tile([128, NCLS], F32, tag="lg", bufs=2)
                for mi in range(NMAT):
                    nc.tensor.matmul(lg[:], matsT[mi][:, t * 128:(t + 1) * 128],
                                     w2_sb[:, mi * NCLS:(mi + 1) * NCLS],
                                     start=(mi == 0), stop=(mi == NMAT - 1))
                dst = logits[:, t * NCLS:(t + 1) * NCLS]
                if b2_sb is not None:
                    nc.vector.tensor_tensor(dst, lg[:], b2_sb[:],
                                            op=mybir.AluOpType.add)
                else:
                    nc.vector.tensor_copy(dst, lg[:])

            # ---- hops ----
            no_cc = os.environ.get("MIXHOP_NO_CC", "0") == "1"
            no_gather = os.environ.get("MIXHOP_NO_GATHER", "0") == "1"
            nsplit = int(os.environ.get("MIXHOP_NSPLIT", "2"))
            # sub-0 gather groups issued ahead of each sub-1 collective:
            # must stay < gp bufs or the in-order gpsimd queue deadlocks on
            # G-buffer reuse semaphores.
            PRE = int(os.environ.get("MIXHOP_PRE", "7"))
            AG_EMIT_G = 10  # hop-0 group after which AG(1,0) is emitted
            MAXGS = int(max(NS0g.max(), NS1g.max()))

            def allgather(hop, s):
                if no_cc:
                    nc.sync.dma_start(out=gout[hop][s][0:QROWS, :],
                                      in_=gin[hop][s][:])
                else:
                    nc.gpsimd.collective_compute(
                        "AllGather", mybir.AluOpType.bypass,
                        replica_groups=rg,
                        ins=[gin[hop][s][:]], outs=[gout[hop][s][:]])

            gtiles = {}

            def gather_sub(hop, g, s):
                """Issue the dma_gather calls for (group g, sub s)."""
                G = gp.tile([128, MAXGS * 128], F16, tag="G")
                gtiles[(hop, g, s)] = G
                G3 = G[:].rearrange("p (c e) -> p c e", e=128)
                ns = int(NS0g[g]) if s == 0 else int(NS1g[g])
                if ns == 0:
                    return
                base = (int(gbase[g]) + (0 if s == 0 else int(NS0g[g]))) \
                    * CHUNK
                if no_gather:
                    nc.vector.memset(G[:, :ns * 128], 0.0)
                    return
                table = gout[hop][s]
                q, r = divmod(ns, nsplit)
                off = 0
                for i in range(nsplit):
                    ncols = q + (1 if i < r else 0)
                    if not ncols:
                        continue
                    s0 = base + off * CHUNK
                    nc.gpsimd.dma_gather(
                        G3[:, off:off + ncols, :], table[:],
                        idx_sb[:, s0 // 16:(s0 + ncols * CHUNK) // 16],
                        ncols * CHUNK, ncols * CHUNK, HID,
                        single_packet=False,
                        queue_num=(s * nsplit + i) % 4)
                    off += ncols

            # the sub-1 collective is emitted after only two sub-0 gathers:
            # those dispatch quickly on fresh rings, so the collective's
            # trigger lands as soon as its gin data is ready instead of
            # queueing behind ring-stalled gather instructions.
            allgather(0, 0)
            for g in range(2):
                gather_sub(0, g, 0)
            allgather(0, 1)
            for g in range(2, PRE):
                gather_sub(0, g, 0)
            for hop in range(2):
                for g in range(NG):
                    if g >= PRE:
                        gather_sub(hop, g, 0)
                    gather_sub(hop, g, 1)
                    G0 = gtiles[(hop, g, 0)]
                    G0_3 = G0[:].rearrange("p (c e) -> p c e", e=128)
                    G1 = gtiles[(hop, g, 1)]
                    G1_3 = G1[:].rearrange("p (c e) -> p c e", e=128)
                    for w in range(g * WG, (g + 1) * WG):
                        cw = int(CW[w])
                        S = sp.tile([128, CMAXW * 128], FP8, tag="S")
                        nc.sync.dma_start(
                            out=S[:, :cw * 128],
                            in_=sdat_d[:, int(soff[w]):int(soff[w])
                                       + cw * 128])
                        acc = pp.tile([128, HID], F32, tag="acc", bufs=4)
                        # self-loop contribution: acc = I @ g_prev[window w]
                        nc.tensor.matmul(acc[:], id_sb[:],
                                         gtsb[:, w * HID:(w + 1) * HID],
                                         start=True, stop=(cw == 0))
                        ci = 0
                        for c in range(int(C0[w])):
                            nc.tensor.matmul(
                                acc[:], S[:, ci * 128:(ci + 1) * 128],
                                G0_3[:, int(col0[w]) + c, :],
                                start=False, stop=(ci == cw - 1))
                            ci += 1
                        for c in range(int(C1[w])):
                            nc.tensor.matmul(
                                acc[:], S[:, ci * 128:(ci + 1) * 128],
                                G1_3[:, int(col1[w]) + c, :],
                                start=False, stop=(ci == cw - 1))
                            ci += 1
                        drain_window(acc, w, hop)
                        if hop == 1:
                            lin2_tile(w)
                    if hop == 0 and g == AG_EMIT_G:
                        allgather(1, 0)
                    if hop == 0 and g == NG - 1:
                        for g2 in range(2):
                            gather_sub(1, g2, 0)
                        allgather(1, 1)
                        for g2 in range(2, PRE):
                            gather_sub(1, g2, 0)

            # ---- log_softmax (lin2 already streamed into hop-1 drains;
            # `final` doubles as the exp scratch before the subtract) ----
            nc.scalar.activation(final[:], logits[:],
                                 mybir.ActivationFunctionType.Exp)
            nc.vector.reduce_sum(
                ssum[:], final[:].rearrange("p (t c) -> p t c", c=NCLS),
                axis=mybir.AxisListType.X)
            nc.scalar.activation(lsum[:], ssum[:],
                                 mybir.ActivationFunctionType.Ln)
            for t in range(NW):
                nc.vector.tensor_scalar_sub(
                    final[:, t * NCLS:(t + 1) * NCLS],
                    logits[:, t * NCLS:(t + 1) * NCLS], lsum[:, t:t + 1])
            nc.sync.dma_start(
                out=y_d[:].rearrange("(t q) c -> q t c", q=128),
                in_=final[:].rearrange("p (t c) -> p t c", c=NCLS))
        perm.release()
        dramp.release()
    nc.compile()
    return nc


def _ensure_ntff_hook():
    """The agent image's antenv lacks axon_hooks; synthesize it so
    run_bass_kernel_spmd(trace=True) can NTFF-profile via the axon .so."""
    import types

    if "antenv.axon_hooks" in sys.modules:
        return
    try:
        from trn_agent_boot.trn_boot import _ntff_profile_via_ctypes
        hook = _ntff_profile_via_ctypes("/opt/axon/libaxon_pjrt.so")
    except Exception:
        hook = None
    mod = types.ModuleType("antenv.axon_hooks")
    mod.get_axon_ntff_profile_hook = lambda: hook
    mod.set_axon_ntff_profile_hook = lambda h: None
    sys.modules["antenv.axon_hooks"] = mod


def kernel(x, edge_index, W1, b1, W2, b2):
    global LAST_EXEC_NS, LAST_RESULTS
    plan, in_maps = _preprocess(x, edge_index, W1, b1, W2, b2)
    nc = _build(plan)
    trace = os.environ.get("MIXHOP_TRACE", "0") == "1"
    if trace:
        _ensure_ntff_hook()
    res = run_bass_kernel_spmd(nc, in_maps, list(range(N_CORES)), trace=trace)
    LAST_EXEC_NS = res.exec_time_ns
    LAST_RESULTS = res
    out = np.concatenate([res.results[p]["y"] for p in range(N_CORES)], axis=0)
    return out.astype(np.float32)


# revision 23
# speedup vs baseline: 1.5755x; 1.5694x over previous
"""Trainium2 Bass kernel for 2-hop MixHop GCN (nn_Mixhop).

Strategy (8 NeuronCores, node sharding):
  h = x @ W1 (+b1);  GCN norm folded into row scales:
      g = dinv * h;  y[d] = dinv[d] * sum_{e: src->d} g[src]
  Per hop the fp16 g-table is assembled with TWO sub-AllGathers (each core
  contributes 4096 rows per sub) so each collective overlaps with compute
  and each 32768-row sub-table is addressable with int16 gather indices.
  Self-loop edges never enter the gather stream: their contribution is a
  per-window identity matmul against an SBUF-resident copy of g (gtsb).
  Remaining edges: per-edge dma_gather of source rows, segment-sum via PE
  matmuls with host-built one-hot fp8 "S" matrices (PSUM accumulation per
  128-dst window).  relu'd mats are PE-transposed into matsT for the final
  lin2 (@W2) + log_softmax.
"""

import os
import sys

sys.path.insert(0, "/opt/trn_rl_repo")

import numpy as np

import concourse.bacc as bacc
import concourse.bass as bass
import concourse.mybir as mybir
import concourse.tile as tile
from concourse.bass_utils import run_bass_kernel_spmd

F32 = mybir.dt.float32
F16 = mybir.dt.float16
FP8 = mybir.dt.float8e4
I16 = mybir.dt.int16
NP_FP8 = mybir.dt.np(FP8)
NP_F16 = np.float16

N_CORES = 8
WIN = 128          # dst nodes per PSUM window
CHUNK = 128        # edges per matmul chunk
WG = 4             # windows per gather group
NSUB = 2           # sub-AllGathers per hop (sub-table = NN/NSUB rows)

LAST_EXEC_NS = None
LAST_RESULTS = None


def _preprocess(x, edge_index, W1, b1, W2, b2):
    """Build the chunk plan (program-level constants, max over cores) and
    per-core input arrays."""
    n_nodes, d_in = x.shape
    hid = W1.shape[1]
    ncls = W2.shape[1]
    nmat = W2.shape[0] // hid
    assert n_nodes % (N_CORES * WIN) == 0
    NLOC = n_nodes // N_CORES
    NW = NLOC // WIN
    assert NW % WG == 0
    NG = NW // WG
    KIN = d_in // 128
    assert d_in % 128 == 0 and hid == 128
    QROWS = NLOC // NSUB           # local rows contributed per sub-AG
    SUBN = n_nodes // NSUB         # rows per assembled sub-table
    assert SUBN <= 32768           # int16 index range

    # data edges all flow through the gather path (including any src==dst
    # pairs in the data); only the synthetic self-loops from _gcn_norm are
    # handled by the per-window identity matmul against gtsb.
    src = np.asarray(edge_index[0], dtype=np.int64)
    dst = np.asarray(edge_index[1], dtype=np.int64)

    deg = (np.bincount(dst, minlength=n_nodes) + 1).astype(np.float32)
    dinv = (1.0 / np.sqrt(deg)).astype(np.float32)

    core = dst // NLOC
    w_of = (dst % NLOC) // WIN
    sub_of = (src % NLOC) // QROWS
    tidx = (src // NLOC) * QROWS + (src % QROWS)
    dloc = (dst % WIN).astype(np.int64)

    # Chunks are packed densely at (group, sub) granularity: each core's
    # edges for a (g, s) unit are sorted by destination window and packed
    # back-to-back into 128-slot chunks, so ceil-padding is paid once per
    # unit instead of once per window.  A chunk near a window boundary then
    # feeds two windows; the program-level matmul range [A, B) per
    # (g, s, window) covers every core's span of that window's chunks (the
    # per-core S pieces are zero where a chunk's slots belong to another
    # window).
    g_of = w_of // WG
    wi_of = w_of % WG
    key4 = ((core * NG + g_of) * NSUB + sub_of) * WG + wi_of
    cnt4 = np.bincount(key4, minlength=N_CORES * NG * NSUB * WG)
    cnt4 = cnt4.reshape(N_CORES, NG, NSUB, WG)
    hi = np.cumsum(cnt4, axis=3)                  # rank end per window
    lo = hi - cnt4                                # rank start per window
    cnt_gs = cnt4.sum(axis=3)                     # [P, NG, NSUB]
    NC = (-(-cnt_gs.max(axis=0) // CHUNK)).astype(np.int64)  # [NG, NSUB]
    A = (lo.min(axis=0) // CHUNK).astype(np.int64)           # [NG, NSUB, WG]
    B = (-(-hi.max(axis=0) // CHUNK)).astype(np.int64)
    B = np.minimum(B, NC[:, :, None])

    SLOTB = np.zeros((NG, NSUB), np.int64)        # slot base per unit
    flat_nc = NC.reshape(-1)
    SLOTB.reshape(-1)[:] = (np.concatenate([[0], np.cumsum(flat_nc)[:-1]])
                            * CHUNK)
    TOTSLOTS = int(NC.sum()) * CHUNK
    MAXGS = int(NC.max())

    PW = (B - A)                                  # pieces per (g, s, w)
    P0 = PW[:, 0, :].reshape(-1)                  # [NW] sub-0 pieces
    CW = PW.sum(axis=1).reshape(-1)               # [NW] total pieces
    soff = np.concatenate([[0], np.cumsum(CW)[:-1]])  # piece-col base (chunks)
    TOTC = int(CW.sum())
    CMAXW = int(CW.max())

    one_fp8 = np.float32(1.0).astype(NP_FP8).view(np.uint8)

    plan = dict(
        n_nodes=n_nodes, NLOC=NLOC, QROWS=QROWS, SUBN=SUBN,
        NW=NW, NG=NG, KIN=KIN,
        hid=hid, ncls=ncls, nmat=nmat,
        NC=NC, A=A, B=B, SLOTB=SLOTB, CW=CW, soff=soff,
        TOTC=TOTC, TOTSLOTS=TOTSLOTS, MAXGS=MAXGS, CMAXW=CMAXW,
        has_b1=bool(np.any(b1 != 0)), has_b2=bool(np.any(b2 != 0)),
    )

    in_maps = []
    for p in range(N_CORES):
        sel = core == p
        w_p, sub_p, dl_p, t_p = (w_of[sel], sub_of[sel], dloc[sel],
                                 tidx[sel])
        g_p, wi_p = w_p // WG, w_p % WG
        k = (g_p * NSUB + sub_p) * WG + wi_p
        order = np.argsort(k, kind="stable")
        ks = k[order]
        # rank within the (g, s) unit (across its windows)
        kgs = ks // WG
        gcnt = np.bincount(kgs, minlength=NG * NSUB)
        run_start = np.cumsum(gcnt) - gcnt
        rank = np.arange(len(kgs)) - np.repeat(run_start, gcnt)
        slots = SLOTB.reshape(-1)[kgs] + rank

        idx_flat = np.zeros(TOTSLOTS, np.int16)
        idx_flat[slots] = t_p[order].astype(np.int16)
        idx16 = idx_flat.reshape(TOTSLOTS // 16, 16).T  # [16, S/16]
        idx_arr = np.tile(idx16, (8, 1)).copy()         # [128, S/16]

        # S pieces: row = pos-in-chunk, col = window-piece-col
        su8 = np.zeros((CHUNK, TOTC * CHUNK), np.uint8)
        c_in = rank // CHUNK
        pos = rank % CHUNK
        w_o = w_p[order]
        sub_o = sub_p[order]
        a_o = A.reshape(-1, WG)[kgs, wi_p[order]]
        scol = ((soff[w_o] + np.where(sub_o == 1, P0[w_o], 0)
                 + (c_in - a_o)) * CHUNK + dl_p[order])
        su8[pos, scol] = one_fp8
        s_arr = su8.view(NP_FP8)

        x_p = np.asarray(x[p * NLOC:(p + 1) * NLOC], dtype=np.float32)
        xt = np.ascontiguousarray(
            x_p.reshape(NW, 128, KIN, 128).transpose(0, 3, 2, 1)
            .reshape(NW, 128, KIN * 128))
        dinv_p = np.ascontiguousarray(
            dinv[p * NLOC:(p + 1) * NLOC].reshape(NW, 128).T)

        m = {
            "xt": xt.astype(NP_F16),
            "w1": np.ascontiguousarray(
                np.asarray(W1, np.float32).reshape(KIN, 128, hid)
                .transpose(1, 0, 2).reshape(128, KIN * hid)).astype(NP_F16),
            "w2": np.ascontiguousarray(
                np.asarray(W2, np.float32).reshape(nmat, hid, ncls)
                .astype(NP_F16).transpose(1, 0, 2).reshape(hid, nmat * ncls)),
            "dinv": dinv_p,
            "dinv2": (dinv_p * dinv_p),
            "idx": idx_arr,
            "sdat": s_arr,
            "ident": np.eye(128, dtype=NP_F16),
        }
        if plan["has_b1"]:
            m["b1bc"] = np.tile(np.asarray(b1, np.float32)[None, :], (128, 1))
        if plan["has_b2"]:
            m["b2bc"] = np.tile(np.asarray(b2, np.float32)[None, :], (128, 1))
        in_maps.append(m)
    return plan, in_maps


def _build(plan):
    P = plan
    NLOC, NW, NG, KIN = P["NLOC"], P["NW"], P["NG"], P["KIN"]
    HID, NCLS, NMAT = P["hid"], P["ncls"], P["nmat"]
    QROWS, SUBN = P["QROWS"], P["SUBN"]
    NC, A, B, SLOTB = P["NC"], P["A"], P["B"], P["SLOTB"]
    CW, soff = P["CW"], P["soff"]
    MAXGS, CMAXW, TOTC, TOTSLOTS = (P["MAXGS"], P["CMAXW"], P["TOTC"],
                                    P["TOTSLOTS"])
    WPS = NW // NSUB   # windows per sub (drain target ranges)

    nc = bacc.Bacc("TRN2", target_bir_lowering=False, debug=False,
                   num_devices=N_CORES, num_swdge_queues=4)
    xt_d = nc.dram_tensor("xt", [NW, 128, KIN * 128], F16,
                          kind="ExternalInput")
    w1_d = nc.dram_tensor("w1", [128, KIN * HID], F16, kind="ExternalInput")
    w2_d = nc.dram_tensor("w2", [128, NMAT * NCLS], F16, kind="ExternalInput")
    dinv_d = nc.dram_tensor("dinv", [128, NW], F32, kind="ExternalInput")
    dinv2_d = nc.dram_tensor("dinv2", [128, NW], F32, kind="ExternalInput")
    idx_d = nc.dram_tensor("idx", [128, TOTSLOTS // 16], I16,
                           kind="ExternalInput")
    sdat_d = nc.dram_tensor("sdat", [128, TOTC * CHUNK], FP8,
                            kind="ExternalInput")
    id_d = nc.dram_tensor("ident", [128, 128], F16, kind="ExternalInput")
    b1_d = (nc.dram_tensor("b1bc", [128, HID], F32, kind="ExternalInput")
            if P["has_b1"] else None)
    b2_d = (nc.dram_tensor("b2bc", [128, NCLS], F32, kind="ExternalInput")
            if P["has_b2"] else None)
    y_d = nc.dram_tensor("y", [NLOC, NCLS], F32, kind="ExternalOutput")

    rg = [list(range(N_CORES))]

    with tile.TileContext(nc) as tc:
        # ---- persistent tiles ----
        perm = tc.alloc_tile_pool(name="perm", bufs=1)
        dramp = tc.alloc_tile_pool(name="dramp", bufs=1, space="DRAM")
        w1_sb = perm.tile([128, KIN * HID], F16, name="w1sb")
        w2_sb = perm.tile([128, NMAT * NCLS], F16, name="w2sb")
        dinv_sb = perm.tile([128, NW], F32, name="dinvsb")
        dinv2_sb = perm.tile([128, NW], F32, name="dinv2sb")
        idx_sb = perm.tile([128, TOTSLOTS // 16], I16, name="idxsb")
        id_sb = perm.tile([128, 128], F16, name="idsb")
        matsT = [perm.tile([128, NLOC], F16, name=f"matsT{i}")
                 for i in range(NMAT)]
        gtsb = perm.tile([128, NW * HID], F16, name="gtsb")
        logits = perm.tile([128, NW * NCLS], F32, name="logits")
        ssum = perm.tile([128, NW], F32, name="ssum")
        lsum = perm.tile([128, NW], F32, name="lsum")
        final = perm.tile([128, NW * NCLS], F32, name="final")
        b1_sb = perm.tile([128, HID], F32, name="b1sb") if b1_d else None
        b2_sb = perm.tile([128, NCLS], F32, name="b2sb") if b2_d else None

        gin = [[dramp.tile([QROWS, HID], F16, name=f"gin{h}_{s}")
                for s in range(NSUB)] for h in range(2)]
        gout = [[dramp.tile([SUBN, HID], F16, addr_space="Shared",
                            name=f"gout{h}_{s}")
                 for s in range(NSUB)] for h in range(2)]

        nc.sync.dma_start(out=w1_sb[:], in_=w1_d[:])
        nc.sync.dma_start(out=w2_sb[:], in_=w2_d[:])
        nc.sync.dma_start(out=dinv_sb[:], in_=dinv_d[:])
        nc.sync.dma_start(out=dinv2_sb[:], in_=dinv2_d[:])
        nc.sync.dma_start(out=idx_sb[:], in_=idx_d[:])
        nc.sync.dma_start(out=id_sb[:], in_=id_d[:])
        if b1_d is not None:
            nc.sync.dma_start(out=b1_sb[:], in_=b1_d[:])
        if b2_d is not None:
            nc.sync.dma_start(out=b2_sb[:], in_=b2_d[:])

        with (
            tc.tile_pool(name="xp", bufs=6) as xp,
            tc.tile_pool(name="gp", bufs=8) as gp,
            tc.tile_pool(name="sp", bufs=8) as sp,
            tc.tile_pool(name="dp", bufs=6) as dp,
            tc.tile_pool(name="pp", bufs=1, space="PSUM") as pp,
        ):
            ACT = mybir.ActivationFunctionType

            def drain_window(acc, w, hop):
                """acc: PSUM [128, HID] f32 for window w; hop 0/1/-1 (lin1).

                lin1: h = acc.  hops: h = dinv * acc (acc includes the self
                term via the identity matmul).  g for the next hop's table
                goes to gtsb (SBUF, feeds the self matmul) and to gin (DRAM,
                feeds the sub-AllGather).  mats = relu(h).  All on ScalarE:
                DVE shares an SBUF port with GpSimd (SWDGE) and stalls while
                gathers run."""
                if hop < 0:
                    # lin1 runs before any SWDGE traffic: DVE is free, and
                    # using it lets the sub-AllGathers launch sooner.
                    gt = gtsb[:, w * HID:(w + 1) * HID]
                    nc.vector.tensor_scalar_mul(gt, acc[:],
                                                dinv_sb[:, w:w + 1])
                    nc.sync.dma_start(
                        out=gin[0][w // WPS][(w % WPS) * 128:
                                             (w % WPS + 1) * 128, :],
                        in_=gt)
                    m = dp.tile([128, HID], F16, tag="m")
                    nc.vector.tensor_scalar_max(m[:], acc[:], 0.0)
                    tp = pp.tile([128, 128], F16, tag="tp", bufs=2)
                    nc.tensor.transpose(tp[:], m[:], id_sb[:])
                    nc.vector.tensor_copy(
                        matsT[0][:, w * 128:(w + 1) * 128], tp[:])
                    return
                hscale = dinv_sb[:, w:w + 1]
                if hop < 1:  # produce g for the next hop
                    gt = gtsb[:, w * HID:(w + 1) * HID]
                    nc.scalar.activation(gt, acc[:], ACT.Copy,
                                         scale=dinv2_sb[:, w:w + 1])
                    s = w // WPS
                    r = w % WPS
                    nc.sync.dma_start(
                        out=gin[hop + 1][s][r * 128:(r + 1) * 128, :], in_=gt)
                m = dp.tile([128, HID], F16, tag="m")
                nc.scalar.activation(m[:], acc[:], ACT.Relu, scale=hscale)
                tp = pp.tile([128, 128], F16, tag="tp", bufs=2)
                nc.tensor.transpose(tp[:], m[:], id_sb[:])
                nc.scalar.activation(matsT[hop + 1][:, w * 128:(w + 1) * 128],
                                     tp[:], ACT.Copy)

            # ---- lin1 ----
            for t in range(NW):
                acc = pp.tile([128, HID], F32, tag="acc", bufs=4)
                xtile = xp.tile([128, KIN * 128], F16, tag="xt")
                nc.sync.dma_start(out=xtile[:], in_=xt_d[t])
                for k in range(KIN):
                    nc.tensor.matmul(acc[:], xtile[:, k * 128:(k + 1) * 128],
                                     w1_sb[:, k * HID:(k + 1) * HID],
                                     start=(k == 0), stop=(k == KIN - 1))
                if b1_sb is not None:
                    hb = dp.tile([128, HID], F32, tag="hb")
                    nc.vector.tensor_tensor(hb[:], acc[:], b1_sb[:],
                                            op=mybir.AluOpType.add)
                    drain_window(hb, t, -1)
                else:
                    drain_window(acc, t, -1)

            def lin2_tile(t):
                lg = pp.tile([128, NCLS], F32, tag="lg", bufs=2)
                for mi in range(NMAT):
                    nc.tensor.matmul(lg[:], matsT[mi][:, t * 128:(t + 1) * 128],
                                     w2_sb[:, mi * NCLS:(mi + 1) * NCLS],
                                     start=(mi == 0), stop=(mi == NMAT - 1))
                dst = logits[:, t * NCLS:(t + 1) * NCLS]
                if b2_sb is not None:
                    nc.vector.tensor_tensor(dst, lg[:], b2_sb[:],
                                            op=mybir.AluOpType.add)
                else:
                    nc.vector.tensor_copy(dst, lg[:])

            # ---- hops ----
            no_cc = os.environ.get("MIXHOP_NO_CC", "0") == "1"
            no_gather = os.environ.get("MIXHOP_NO_GATHER", "0") == "1"
            nsplit = int(os.environ.get("MIXHOP_NSPLIT", "2"))
            # sub-0 gather groups issued ahead of each sub-1 collective:
            # must stay < gp bufs or the in-order gpsimd queue deadlocks on
            # G-buffer reuse semaphores.
            PRE = int(os.environ.get("MIXHOP_PRE", "7"))
            AG_EMIT_G = 10  # hop-0 group after which AG(1,0) is emitted

            def allgather(hop, s):
                if no_cc:
                    nc.sync.dma_start(out=gout[hop][s][0:QROWS, :],
                                      in_=gin[hop][s][:])
                else:
                    nc.gpsimd.collective_compute(
                        "AllGather", mybir.AluOpType.bypass,
                        replica_groups=rg,
                        ins=[gin[hop][s][:]], outs=[gout[hop][s][:]])

            gtiles = {}

            def gather_sub(hop, g, s):
                """Issue the dma_gather calls for (group g, sub s)."""
                G = gp.tile([128, MAXGS * 128], F16, tag="G")
                gtiles[(hop, g, s)] = G
                G3 = G[:].rearrange("p (c e) -> p c e", e=128)
                ns = int(NC[g, s])
                if ns == 0:
                    return
                base = int(SLOTB[g, s])
                if no_gather:
                    nc.vector.memset(G[:, :ns * 128], 0.0)
                    return
                table = gout[hop][s]
                q, r = divmod(ns, nsplit)
                off = 0
                for i in range(nsplit):
                    ncols = q + (1 if i < r else 0)
                    if not ncols:
                        continue
                    s0 = base + off * CHUNK
                    nc.gpsimd.dma_gather(
                        G3[:, off:off + ncols, :], table[:],
                        idx_sb[:, s0 // 16:(s0 + ncols * CHUNK) // 16],
                        ncols * CHUNK, ncols * CHUNK, HID,
                        single_packet=False,
                        queue_num=(s * nsplit + i) % 4)
                    off += ncols

            # the sub-1 collective is emitted after only two sub-0 gathers:
            # those dispatch quickly on fresh rings, so the collective's
            # trigger lands as soon as its gin data is ready instead of
            # queueing behind ring-stalled gather instructions.
            allgather(0, 0)
            for g in range(2):
                gather_sub(0, g, 0)
            allgather(0, 1)
            for g in range(2, PRE):
                gather_sub(0, g, 0)
            for hop in range(2):
                for g in range(NG):
                    if g >= PRE:
                        gather_sub(hop, g, 0)
                    gather_sub(hop, g, 1)
                    G0 = gtiles[(hop, g, 0)]
                    G0_3 = G0[:].rearrange("p (c e) -> p c e", e=128)
                    G1 = gtiles[(hop, g, 1)]
                    G1_3 = G1[:].rearrange("p (c e) -> p c e", e=128)
                    for w in range(g * WG, (g + 1) * WG):
                        wi = w % WG
                        cw = int(CW[w])
                        S = sp.tile([128, CMAXW * 128], FP8, tag="S")
                        nc.sync.dma_start(
                            out=S[:, :cw * 128],
                            in_=sdat_d[:, int(soff[w]) * 128:
                                       (int(soff[w]) + cw) * 128])
                        acc = pp.tile([128, HID], F32, tag="acc", bufs=4)
                        # self-loop contribution: acc = I @ g_prev[window w]
                        nc.tensor.matmul(acc[:], id_sb[:],
                                         gtsb[:, w * HID:(w + 1) * HID],
                                         start=True, stop=(cw == 0))
                        ci = 0
                        for c in range(int(A[g, 0, wi]), int(B[g, 0, wi])):
                            nc.tensor.matmul(
                                acc[:], S[:, ci * 128:(ci + 1) * 128],
                                G0_3[:, c, :],
                                start=False, stop=(ci == cw - 1))
                            ci += 1
                        for c in range(int(A[g, 1, wi]), int(B[g, 1, wi])):
                            nc.tensor.matmul(
                                acc[:], S[:, ci * 128:(ci + 1) * 128],
                                G1_3[:, c, :],
                                start=False, stop=(ci == cw - 1))
                            ci += 1
                        drain_window(acc, w, hop)
                        if hop == 1:
                            lin2_tile(w)
                            # stream the softmax exp/rowsum for window w
                            fw = final[:, w * NCLS:(w + 1) * NCLS]
                            nc.scalar.activation(
                                fw, logits[:, w * NCLS:(w + 1) * NCLS],
                                ACT.Exp)
                            nc.vector.reduce_sum(
                                ssum[:, w:w + 1],
                                fw.rearrange("p (t c) -> p t c", c=NCLS),
                                axis=mybir.AxisListType.X)
                    if hop == 0 and g == AG_EMIT_G:
                        allgather(1, 0)
                    if hop == 0 and g == NG - 1:
                        for g2 in range(2):
                            gather_sub(1, g2, 0)
                        allgather(1, 1)
                        for g2 in range(2, PRE):
                            gather_sub(1, g2, 0)

            # ---- log_softmax tail (exp/rowsum streamed per window above;
            # `final` doubled as the exp scratch before the subtract) ----
            nc.scalar.activation(lsum[:], ssum[:],
                                 mybir.ActivationFunctionType.Ln)
            for t in range(NW):
                nc.vector.tensor_scalar_sub(
                    final[:, t * NCLS:(t + 1) * NCLS],
                    logits[:, t * NCLS:(t + 1) * NCLS], lsum[:, t:t + 1])
            nc.sync.dma_start(
                out=y_d[:].rearrange("(t q) c -> q t c", q=128),
                in_=final[:].rearrange("p (t c) -> p t c", c=NCLS))
        perm.release()
        dramp.release()
    nc.compile()
    return nc


def _ensure_ntff_hook():
    """The agent image's antenv lacks axon_hooks; synthesize it so
    run_bass_kernel_spmd(trace=True) can NTFF-profile via the axon .so."""
    import types

    if "antenv.axon_hooks" in sys.modules:
        return
    try:
        from trn_agent_boot.trn_boot import _ntff_profile_via_ctypes
        hook = _ntff_profile_via_ctypes("/opt/axon/libaxon_pjrt.so")
    except Exception:
        hook = None
    mod = types.ModuleType("antenv.axon_hooks")
    mod.get_axon_ntff_profile_hook = lambda: hook
    mod.set_axon_ntff_profile_hook = lambda h: None
    sys.modules["antenv.axon_hooks"] = mod


def kernel(x, edge_index, W1, b1, W2, b2):
    global LAST_EXEC_NS, LAST_RESULTS
    plan, in_maps = _preprocess(x, edge_index, W1, b1, W2, b2)
    nc = _build(plan)
    trace = os.environ.get("MIXHOP_TRACE", "0") == "1"
    if trace:
        _ensure_ntff_hook()
    res = run_bass_kernel_spmd(nc, in_maps, list(range(N_CORES)), trace=trace)
    LAST_EXEC_NS = res.exec_time_ns
    LAST_RESULTS = res
    out = np.concatenate([res.results[p]["y"] for p in range(N_CORES)], axis=0)
    return out.astype(np.float32)
